# revision 1
# baseline (speedup 1.0000x reference)
"""Trainium2 Bass kernel for nn_EntropyBasedLossBase (joint-KDE-histogram entropies).

Sharding: data parallel over batch B=8 across 8 NeuronCores (one sample-row
pair per core, N=131072 reshaped to [128 partitions, 1024]).

Algorithm (per core):
1. The Parzen/xu expansion vals[n,i] = k(i - s_n) has exactly two nonzero
   entries per sample: w = k(f) at bin b = floor(s) and 1-w at b+1 (since
   k(f) + k(1-f) == 1). Its cumulative sum along bins is a clamped staircase
       S[n,i] = clamp(i+1 - z'_n, 0, 1),  z' = b + 1 - k(s-b) = s + 0.9u - 1.8u|u|
   with u = frac(s) - 0.5 -- z' is a single compact per-sample value
   (computed with a float32 magic-number round and a fused ALU chain).
2. Expansion to [128 samples, 64 bins] tiles is 2 DVE ops per 64-chunk group:
   a broadcast-AP scalar_tensor_tensor (iota - z') and a fused clamp
   tensor_scalar, in fp16. Signal 2 is built as the ANTI-staircase
   S2b = 1 - S2 = clamp(z2 - i, 0, 1) via two ScalarE Relu ops, moving that
   work off the bottleneck DVE.
3. PE computes Mt = S1^T S2b in fp16, two 128-sample chunks per matmul
   (128-col weights -> FWL), rhs carries 2 extra ones-columns so the same
   matmuls also accumulate R1 = sum_n S1[n,:]. Accumulation is split over 6
   PSUM tiles for fp32 precision.
4. The joint histogram is recovered on the tiny 64x64 output:
   joint = (D R1) e0^T - D coldiff(Mt) D^T   (D = first-difference matrix),
   then clip(eps), marginals, and H = ln(T) - sum(c ln c)/T on-device
   (ScalarE Ln + GPSIMD partition reductions). Output: 3 scalars per core.
"""
import sys

sys.path.insert(0, "/opt/trn_rl_repo")

from contextlib import ExitStack

import numpy as np

import concourse.bacc as bacc
import concourse.bass as bass
import concourse.bass_isa as bass_isa
import concourse.tile as tile
from concourse import mybir
from concourse.bass_utils import run_bass_kernel_spmd

F32 = mybir.dt.float32
F16 = mybir.dt.float16
OP = mybir.AluOpType
ACT = mybir.ActivationFunctionType

NB = 64            # num bins
P = 128            # partitions
NCOL = 1024        # free dim of the compact [128, 1024] layout (N = P*NCOL)
GCH = 32           # chunks per expansion group -> [128, GCH*64] tiles
NGROUP = NCOL // GCH   # 32 groups
NPSUM = 6          # split M accumulation over several psum tiles (precision + ILP)
EPS = float(np.finfo(np.float32).eps)
MAGIC = 12582912.0  # 1.5 * 2^23: float32 round-to-int shift constant


def _bcast_ap(t, col0, ncols, reps):
    """View t[:, col0:col0+ncols] ([128, ncols]) as [128, ncols, reps] with the
    last dim broadcast (step 0)."""
    ap = t[:, col0:col0 + ncols]
    return bass.AP(ap.tensor, ap.offset, [ap.ap[0], ap.ap[1], [0, reps]])


def build_nc(repeat=1, do_mm=True, do_exp=True, act_groups=16, gps_groups=0, no_clamp=False, dense_stt=False, gch=GCH, comp_bufs=2, chunk_mode=False, act_q=0, anti_both=False, tt16=False):
    GCHl = gch
    NGROUPl = NCOL // GCHl
    nc = bacc.Bacc("TRN2", num_devices=8)

    sig1 = nc.dram_tensor("sig1", [P, NCOL], F32, kind="ExternalInput")
    sig2 = nc.dram_tensor("sig2", [P, NCOL], F32, kind="ExternalInput")
    c_dt = nc.dram_tensor("c_dt", [NB, NB], F32, kind="ExternalInput")
    out_h = nc.dram_tensor("out_h", [1, 4], F32, kind="ExternalOutput")

    NPAIR = GCHl // 2          # matmul chunk pairs per group
    RW = 2 * NB + 2           # rhs width per pair (128 S cols + 2 ones cols)

    with ExitStack() as ctx:
        tc = ctx.enter_context(tile.TileContext(nc))
        eb = 3 if GCHl <= 32 else 2
        singles = ctx.enter_context(tc.tile_pool(name="singles", bufs=1))
        comp = ctx.enter_context(tc.tile_pool(name="comp", bufs=comp_bufs))
        texp = ctx.enter_context(tc.tile_pool(name="texp", bufs=eb))
        sexp = ctx.enter_context(tc.tile_pool(name="sexp", bufs=eb))
        psum = ctx.enter_context(tc.tile_pool(name="psum", bufs=1, space="PSUM"))
        post = ctx.enter_context(tc.tile_pool(name="post", bufs=1))
        postp = ctx.enter_context(tc.tile_pool(name="postp", bufs=1, space="PSUM"))

        # ---- constants ----
        iota1 = singles.tile([P, GCHl * NB], F16)
        nc.gpsimd.iota(iota1[:], pattern=[[0, GCHl], [1, NB]], base=1,
                       channel_multiplier=0, allow_small_or_imprecise_dtypes=True)
        iota0 = singles.tile([P, GCHl * NB], F16)
        nc.gpsimd.iota(iota0[:], pattern=[[0, GCHl], [1, NB]], base=0,
                       channel_multiplier=0, allow_small_or_imprecise_dtypes=True)
        ioneg = singles.tile([P, NB], F16)
        nc.gpsimd.iota(ioneg[:], pattern=[[-1, NB]], base=0,
                       channel_multiplier=0, allow_small_or_imprecise_dtypes=True)
        s2bufs = []
        for sb_i in range(3):
            s2b = singles.tile([P, (GCHl // 2) * (2 * NB + 2)], F16, name=f"s2buf{sb_i}")
            ones_ap = bass.AP(s2b.tensor, s2b.offset + 2 * NB,
                              [s2b.ap[0], [2 * NB + 2, GCHl // 2], [1, 2]])
            nc.vector.memset(ones_ap, 1.0)
            s2bufs.append(s2b)
        dtm = singles.tile([NB, NB], F32)
        nc.sync.dma_start(out=dtm[:], in_=c_dt.ap())
        ones_col = singles.tile([NB, 1], F32)
        nc.vector.memset(ones_col[:], 1.0)
        onesw = singles.tile([P, 1], F16)
        nc.vector.memset(onesw[:], 1.0)

        for _rep in range(repeat):
            # ---- load + per-sample compact pass (both signals, fused) ----
            comb = comp.tile([P, 2 * NCOL], F32, tag="comb")   # s for both signals
            for k, sig in enumerate((sig1, sig2)):
                v = comp.tile([P, NCOL], F32, tag=f"v{k}")
                nc.sync.dma_start(out=v[:], in_=sig.ap())

                mx1 = comp.tile([1, 1], F32, tag=f"mx1{k}")
                mn1 = comp.tile([1, 1], F32, tag=f"mn1{k}")
                nv = comp.tile([P, NCOL], F32, tag=f"nv{k}")
                nc.vector.tensor_scalar(out=nv[:], in0=v[:], scalar1=-1.0, scalar2=None, op0=OP.mult)
                nc.gpsimd.tensor_reduce(out=mx1[:], in_=v[:], axis=mybir.AxisListType.XYZWC, op=OP.max)
                nc.gpsimd.tensor_reduce(out=mn1[:], in_=nv[:], axis=mybir.AxisListType.XYZWC, op=OP.max)
                mxa = comp.tile([P, 1], F32, tag=f"mxa{k}")
                mnn = comp.tile([P, 1], F32, tag=f"mnn{k}")
                nc.gpsimd.partition_broadcast(mxa[:], mx1[:])
                nc.gpsimd.partition_broadcast(mnn[:], mn1[:])
                mna = comp.tile([P, 1], F32, tag=f"mna{k}")
                nc.vector.tensor_scalar(out=mna[:], in0=mnn[:], scalar1=-1.0, scalar2=None, op0=OP.mult)

                diff = comp.tile([P, 1], F32, tag=f"diff{k}")
                nc.vector.tensor_tensor(out=diff[:], in0=mxa[:], in1=mna[:], op=OP.subtract)
                rdiff = comp.tile([P, 1], F32, tag=f"rdiff{k}")
                nc.vector.reciprocal(out=rdiff[:], in_=diff[:])
                guard = comp.tile([P, 1], F32, tag=f"guard{k}")
                nc.vector.tensor_scalar(out=guard[:], in0=diff[:], scalar1=EPS, scalar2=None, op0=OP.is_gt)
                rs = comp.tile([P, 1], F32, tag=f"rs{k}")
                nc.vector.tensor_scalar(out=rs[:], in0=rdiff[:], scalar1=float(NB), scalar2=None, op0=OP.mult)
                nc.vector.tensor_tensor(out=rs[:], in0=rs[:], in1=guard[:], op=OP.mult)

                # s = (v - mn) * rscale in [0, 64]
                nc.vector.tensor_scalar(out=comb[:, k * NCOL:(k + 1) * NCOL], in0=v[:],
                                        scalar1=mna[:], scalar2=rs[:],
                                        op0=OP.subtract, op1=OP.mult)

            # z' = s + 0.9u - 1.8u|u|, u = (s - rhe(s-0.5)) - 0.5, one fused chain
            b1 = comp.tile([P, 2 * NCOL], F32, tag="b1")
            nc.vector.tensor_scalar(out=b1[:], in0=comb[:], scalar1=-0.5, scalar2=MAGIC,
                                    op0=OP.add, op1=OP.add)
            bb = comp.tile([P, 2 * NCOL], F32, tag="bb")
            nc.vector.tensor_scalar(out=bb[:], in0=b1[:], scalar1=-MAGIC, scalar2=None, op0=OP.add)
            nu = comp.tile([P, 2 * NCOL], F32, tag="nu")
            nc.vector.scalar_tensor_tensor(out=nu[:], in0=bb[:], scalar=0.5, in1=comb[:],
                                           op0=OP.add, op1=OP.subtract)  # = -u
            au = comp.tile([P, 2 * NCOL], F32, tag="au")
            nc.scalar.activation(out=au[:], in_=nu[:], func=ACT.Abs)      # = |u|
            ma = comp.tile([P, 2 * NCOL], F32, tag="ma")
            nc.vector.scalar_tensor_tensor(out=ma[:], in0=nu[:], scalar=1.8, in1=au[:],
                                           op0=OP.mult, op1=OP.mult)      # = -1.8 u |u|
            zq = comp.tile([P, 2 * NCOL], F32, tag="zq")
            nc.vector.scalar_tensor_tensor(out=zq[:], in0=nu[:], scalar=-0.9, in1=ma[:],
                                           op0=OP.mult, op1=OP.add)       # = 0.9u - 1.8u|u|
            zc = comp.tile([P, 2 * NCOL], F32, tag="zc")
            nc.vector.tensor_tensor(out=zc[:], in0=comb[:], in1=zq[:], op=OP.add)
            zp = [zc[:, 0:NCOL], zc[:, NCOL:2 * NCOL]]
            if tt16:
                zc16 = comp.tile([P, 2 * NCOL], F16, tag="zc16")
                nc.vector.tensor_copy(out=zc16[:], in_=zc[:])
                zp = [zc16[:, 0:NCOL], zc16[:, NCOL:2 * NCOL]]
            if chunk_mode:
                nzc = comp.tile([P, 2 * NCOL], F32, tag="nzc")
                nc.vector.tensor_scalar(out=nzc[:], in0=zc[:], scalar1=-1.0, scalar2=None,
                                        op0=OP.mult)
                nzp = [nzc[:, 0:NCOL], nzc[:, NCOL:2 * NCOL]]

            if not do_exp:
                hout = post.tile([1, 4], F32, tag="hout_ab")
                nc.vector.memset(hout[:], 0.0)
                nc.sync.dma_start(out=out_h.ap(), in_=hout[:])
                continue

            # ---- expansion + matmul over groups of GCHl chunks ----
            npsum = NPSUM - 1 if anti_both else NPSUM
            mps = []
            for j in range(npsum):
                mtile = psum.tile([P, RW], F32, tag=f"mps{j}", name=f"mps{j}")
                mps.append(mtile)
            c2ps = None
            if anti_both:
                c2ps = psum.tile([1, RW], F32, tag="c2ps", name="c2ps")
            n_mm = NGROUPl * NPAIR
            mm_idx = 0
            for g in range(NGROUPl):
                if chunk_mode:
                    s1t = sexp.tile([P, GCHl * NB], F16, tag="s1t")
                    s2t = s2bufs[g % 3]
                    s2_data_ap = bass.AP(s2t.tensor, s2t.offset,
                                         [s2t.ap[0], [RW, NPAIR], [1, 2 * NB]])
                    ib = iota1[:, 0:NB]
                    for c in range(GCHl):
                        col = g * GCHl + c
                        z1s = zp[0][:, c + g * GCHl:c + g * GCHl + 1]
                        nz1s = nzp[0][:, col:col + 1]
                        o1 = s1t[:, c * NB:(c + 1) * NB]
                        if (c % 8) < act_q:
                            nc.scalar.activation(out=o1, in_=ib, func=ACT.Relu,
                                                 bias=nz1s, scale=1.0)
                        else:
                            nc.vector.tensor_scalar(out=o1, in0=ib, scalar1=z1s,
                                                    scalar2=0.0, op0=OP.subtract, op1=OP.max)
                        # signal 2 anti: relu(z2 - i)
                        m2, r2 = divmod(c, 2)
                        o2 = bass.AP(s2t.tensor, s2t.offset + m2 * RW + r2 * NB,
                                     [s2t.ap[0], [1, NB]])
                        z2s = zp[1][:, col:col + 1]
                        nz2s = nzp[1][:, col:col + 1]
                        if (c % 8) < act_q:
                            nc.scalar.activation(out=o2, in_=ioneg[:], func=ACT.Relu,
                                                 bias=z2s, scale=1.0)
                        else:
                            nc.vector.tensor_scalar(out=o2, in0=ioneg[:], scalar1=nz2s,
                                                    scalar2=0.0, op0=OP.subtract, op1=OP.max)
                    nc.vector.tensor_scalar(out=s1t[:], in0=s1t[:], scalar1=1.0,
                                            scalar2=None, op0=OP.min)
                    nc.vector.tensor_scalar(out=s2_data_ap, in0=s2_data_ap, scalar1=1.0,
                                            scalar2=None, op0=OP.min)
                    if do_mm:
                        for m in range(NPAIR):
                            j = mm_idx % npsum
                            nc.tensor.matmul(
                                out=mps[j][:],
                                lhsT=s1t[:, m * 2 * NB:(m + 1) * 2 * NB],
                                rhs=s2t[:, m * RW:(m + 1) * RW],
                                start=(mm_idx < npsum), stop=(mm_idx >= n_mm - npsum),
                            )
                            mm_idx += 1
                    continue
                # signal 1: staircase (or anti-staircase via ACT when anti_both)
                t1 = texp.tile([P, GCHl * NB], F16, tag="t1")
                if tt16:
                    nc.vector.tensor_tensor(
                        out=t1[:], in0=iota1[:],
                        in1=_bcast_ap(zp[0], g * GCHl, GCHl, NB), op=OP.subtract)
                else:
                    nc.vector.scalar_tensor_tensor(
                        out=t1[:], in0=iota1[:], scalar=0.0,
                        in1=iota1[:] if dense_stt else _bcast_ap(zp[0], g * GCHl, GCHl, NB),
                        op0=OP.bypass, op1=OP.subtract)
                s1t = sexp.tile([P, GCHl * NB], F16, tag="s1t")
                if anti_both:
                    a1 = texp.tile([P, GCHl * NB], F16, tag="a1")
                    nc.scalar.activation(out=a1[:], in_=t1[:], func=ACT.Relu)
                    nc.scalar.activation(out=s1t[:], in_=a1[:], func=ACT.Relu,
                                         bias=1.0, scale=-1.0)
                else:
                    nc.vector.tensor_scalar(out=s1t[:], in0=t1[:], scalar1=0.0, scalar2=1.0,
                                            op0=OP.max, op1=OP.min)
                # signal 2: anti-staircase S2b = 1 - S2 = clamp(z2 - iota0, 0, 1)
                # with persistent ones columns per pair: layout [pair RW=130]
                s2t = s2bufs[g % 3]
                s2_data_ap = bass.AP(s2t.tensor, s2t.offset,
                                     [s2t.ap[0], [RW, NPAIR], [1, 2 * NB]])
                use_act = (g % NGROUPl) < act_groups
                if use_act:
                    t2 = texp.tile([P, GCHl * NB], F16, tag="t2")
                    if tt16:
                        nc.vector.tensor_tensor(
                            out=t2[:], in0=iota1[:],
                            in1=_bcast_ap(zp[1], g * GCHl, GCHl, NB), op=OP.subtract)
                    else:
                        nc.vector.scalar_tensor_tensor(
                            out=t2[:], in0=iota1[:], scalar=0.0,
                            in1=iota1[:] if dense_stt else _bcast_ap(zp[1], g * GCHl, GCHl, NB),
                            op0=OP.bypass, op1=OP.subtract)
                    a2 = texp.tile([P, GCHl * NB], F16, tag="a2")
                    nc.scalar.activation(out=a2[:], in_=t2[:], func=ACT.Relu)
                    nc.scalar.activation(out=s2_data_ap, in_=a2[:], func=ACT.Relu,
                                         bias=1.0, scale=-1.0)
                else:
                    t2 = texp.tile([P, GCHl * NB], F16, tag="t2")
                    nc.vector.scalar_tensor_tensor(
                        out=t2[:], in0=iota0[:], scalar=-1.0,
                        in1=iota0[:] if dense_stt else _bcast_ap(zp[1], g * GCHl, GCHl, NB),
                        op0=OP.mult, op1=OP.add)
                    nc.vector.tensor_scalar(out=s2_data_ap, in0=t2[:], scalar1=0.0, scalar2=1.0,
                                            op0=OP.max, op1=OP.min)
                if do_mm:
                    for m in range(NPAIR):
                        j = mm_idx % npsum
                        nc.tensor.matmul(
                            out=mps[j][:],
                            lhsT=s1t[:, m * 2 * NB:(m + 1) * 2 * NB],
                            rhs=s2t[:, m * RW:(m + 1) * RW],
                            start=(mm_idx < npsum), stop=(mm_idx >= n_mm - npsum),
                        )
                        if anti_both:
                            nc.tensor.matmul(
                                out=c2ps[:],
                                lhsT=onesw[:],
                                rhs=s2t[:, m * RW:(m + 1) * RW],
                                start=(mm_idx == 0), stop=(mm_idx == n_mm - 1),
                            )
                        mm_idx += 1

            if not do_mm:
                hout = post.tile([1, 4], F32, tag="hout_ab")
                nc.vector.memset(hout[:], 0.0)
                nc.sync.dma_start(out=out_h.ap(), in_=hout[:])
                continue

            # ---- combine psum tiles ----
            acc = post.tile([P, RW], F32)
            nc.vector.tensor_copy(out=acc[:], in_=mps[0][:])
            for j in range(1, len(mps)):
                nc.vector.tensor_tensor(out=acc[:], in0=mps[j][:], in1=acc[:], op=OP.add)
            accb = post.tile([NB, NB + 2], F32)
            nc.sync.dma_start(out=accb[:], in_=acc[NB:P, NB:RW])
            # Mt = block(0,0) + block(1,1)   (Mt = S1^T (1 - S2))
            msb = post.tile([NB, NB + 1], F32)
            nc.vector.memset(msb[:, 0:1], 0.0)
            nc.vector.tensor_tensor(out=msb[:, 1:NB + 1], in0=acc[0:NB, 0:NB],
                                    in1=accb[:, 0:NB], op=OP.add)
            # jcr = [coldiff(Mt) | R1]
            jcr = post.tile([NB, NB + 1], F32)
            nc.vector.tensor_tensor(out=jcr[:, 0:NB], in0=msb[:, 1:NB + 1], in1=msb[:, 0:NB],
                                    op=OP.subtract)
            nc.vector.tensor_tensor(out=jcr[:, NB:NB + 1], in0=acc[0:NB, 2 * NB:2 * NB + 1],
                                    in1=accb[:, NB:NB + 1], op=OP.add)
            # [D coldiff(Mt) | D R1]
            jps = postp.tile([NB, NB + 1], F32)
            nc.tensor.matmul(out=jps[:], lhsT=dtm[:], rhs=jcr[:], start=True, stop=True)
            jsb = post.tile([NB, NB], F32)
            if anti_both:
                # joint = D Mh D^T - (D Rb1) e0^T - e0 (D Cb2)^T + Nc e0 e0^T
                cc = post.tile([1, NB], F32)
                nc.vector.tensor_copy(out=cc[:], in_=c2ps[0:1, 0:NB])
                nc.vector.tensor_tensor(out=cc[:], in0=c2ps[0:1, NB:2 * NB], in1=cc[:],
                                        op=OP.add)
                ccp = post.tile([1, NB + 1], F32)
                nc.vector.memset(ccp[:, 0:1], 0.0)
                nc.vector.tensor_copy(out=ccp[:, 1:NB + 1], in_=cc[:])
                dc2 = post.tile([1, NB], F32)
                nc.vector.tensor_tensor(out=dc2[:], in0=ccp[:, 1:NB + 1], in1=ccp[:, 0:NB],
                                        op=OP.subtract)
                nc.vector.tensor_copy(out=jsb[:], in_=jps[:, 0:NB])
                nc.vector.tensor_tensor(out=jsb[:, 0:1], in0=jsb[:, 0:1],
                                        in1=jps[:, NB:NB + 1], op=OP.subtract)
                nc.vector.tensor_tensor(out=jsb[0:1, :], in0=jsb[0:1, :], in1=dc2[:],
                                        op=OP.subtract)
                nc.vector.tensor_scalar(out=jsb[0:1, 0:1], in0=jsb[0:1, 0:1],
                                        scalar1=float(P * NCOL), scalar2=None, op0=OP.add)
            else:
                # joint = (D R1) e0^T - D coldiff(Mt) D^T
                nc.vector.tensor_scalar(out=jsb[:], in0=jps[:, 0:NB], scalar1=-1.0, scalar2=None,
                                        op0=OP.mult)
                nc.vector.tensor_tensor(out=jsb[:, 0:1], in0=jps[:, NB:NB + 1], in1=jsb[:, 0:1],
                                        op=OP.add)

            # ---- clip, sums, entropies ----
            cj = post.tile([NB, NB], F32)
            rowsum = post.tile([NB, 1], F32)
            nc.vector.tensor_scalar(out=cj[:], in0=jsb[:], scalar1=EPS, scalar2=None,
                                    op0=OP.max, op1=OP.add, accum_out=rowsum[:])
            tot = post.tile([NB, 1], F32)
            nc.gpsimd.partition_all_reduce(tot[:], rowsum[:], channels=NB,
                                           reduce_op=bass_isa.ReduceOp.add)

            ly = post.tile([NB, 1], F32)
            nc.scalar.activation(out=ly[:], in_=rowsum[:], func=ACT.Ln)
            cly = post.tile([NB, 1], F32)
            nc.vector.tensor_tensor(out=cly[:], in0=rowsum[:], in1=ly[:], op=OP.mult)
            sy = post.tile([NB, 1], F32)
            nc.gpsimd.partition_all_reduce(sy[:], cly[:], channels=NB,
                                           reduce_op=bass_isa.ReduceOp.add)

            lj = post.tile([NB, NB], F32)
            nc.scalar.activation(out=lj[:], in_=cj[:], func=ACT.Ln)
            clj = post.tile([NB, NB], F32)
            rowsum_cl = post.tile([NB, 1], F32)
            nc.vector.tensor_tensor(out=clj[:], in0=cj[:], in1=lj[:], op=OP.mult)
            nc.vector.tensor_reduce(out=rowsum_cl[:], in_=clj[:], axis=mybir.AxisListType.X, op=OP.add)
            sxy = post.tile([NB, 1], F32)
            nc.gpsimd.partition_all_reduce(sxy[:], rowsum_cl[:], channels=NB,
                                           reduce_op=bass_isa.ReduceOp.add)

            pxp = postp.tile([1, NB], F32)
            nc.tensor.matmul(out=pxp[:], lhsT=ones_col[:], rhs=cj[:], start=True, stop=True)
            px = post.tile([1, NB], F32)
            nc.vector.tensor_copy(out=px[:], in_=pxp[:])
            lx = post.tile([1, NB], F32)
            nc.scalar.activation(out=lx[:], in_=px[:], func=ACT.Ln)
            clx = post.tile([1, NB], F32)
            sx = post.tile([1, 1], F32)
            nc.vector.tensor_tensor(out=clx[:], in0=px[:], in1=lx[:], op=OP.mult)
            nc.vector.tensor_reduce(out=sx[:], in_=clx[:], axis=mybir.AxisListType.X, op=OP.add)

            lnT = post.tile([1, 1], F32)
            nc.scalar.activation(out=lnT[:], in_=tot[0:1, 0:1], func=ACT.Ln)
            rT = post.tile([1, 1], F32)
            nc.vector.reciprocal(out=rT[:], in_=tot[0:1, 0:1])

            hout = post.tile([1, 4], F32)
            for col, sv in ((0, sx[0:1, 0:1]), (1, sy[0:1, 0:1]), (2, sxy[0:1, 0:1])):
                tmp = post.tile([1, 1], F32, tag=f"tmp{col}")
                nc.vector.tensor_tensor(out=tmp[:], in0=sv, in1=rT[:], op=OP.mult)
                nc.vector.tensor_tensor(out=hout[:, col:col + 1], in0=lnT[:], in1=tmp[:],
                                        op=OP.subtract)
            nc.vector.memset(hout[:, 3:4], 0.0)
            nc.sync.dma_start(out=out_h.ap(), in_=hout[:])

    nc.compile()
    return nc


BEST_KW = {"gch": 64, "comp_bufs": 1, "act_groups": 16, "tt16": True}

_NC_CACHE = {}


def _get_nc(repeat=1, **kw):
    key = (repeat, tuple(sorted(kw.items())))
    if key not in _NC_CACHE:
        _NC_CACHE[key] = build_nc(repeat, **kw)
    return _NC_CACHE[key]


def _dt_matrix():
    # c_dt[k, m] = D[m, k] with D = I - subdiag  (joint = D @ coldiff(M))
    d = np.zeros((NB, NB), np.float32)
    for k in range(NB):
        d[k, k] = 1.0
        if k + 1 < NB:
            d[k, k + 1] = -1.0
    return d


def kernel(reference_signal: np.ndarray, other_signal: np.ndarray):
    B, N = reference_signal.shape
    assert (B, N) == (8, 131072)
    nc = _get_nc(1, **BEST_KW)
    c_dt = _dt_matrix()
    in_maps = []
    for r in range(B):
        in_maps.append({
            "sig1": np.ascontiguousarray(reference_signal[r].reshape(P, NCOL)),
            "sig2": np.ascontiguousarray(other_signal[r].reshape(P, NCOL)),
            "c_dt": c_dt,
        })
    res = run_bass_kernel_spmd(nc, in_maps, list(range(8)))
    hx = np.empty(B, np.float32)
    hy = np.empty(B, np.float32)
    hxy = np.empty(B, np.float32)
    for r in range(B):
        o = res.results[r]["out_h"]
        hx[r], hy[r], hxy[r] = o[0, 0], o[0, 1], o[0, 2]
    return (hx, hy, hxy)


def _build_sharded(nc, in_maps):
    """Replicate bass2jax.run_bass_via_pjrt's jit construction, returning a
    callable + prepared args so executions can be repeated/timed."""
    import jax
    import numpy as _np
    from jax.sharding import Mesh, PartitionSpec
    from jax.experimental.shard_map import shard_map
    from concourse import bass2jax as b2j

    b2j.install_neuronx_cc_hook()
    nc_ = nc
    partition_name = nc_.partition_id_tensor.name if nc_.partition_id_tensor else None
    in_names, out_names, out_avals, zero_outs = [], [], [], []
    for alloc in nc_.m.functions[0].allocations:
        if not isinstance(alloc, mybir.MemoryLocationSet):
            continue
        name = alloc.memorylocations[0].name
        if alloc.kind == "ExternalInput":
            if name != partition_name:
                in_names.append(name)
        elif alloc.kind == "ExternalOutput":
            out_names.append(name)
            shape = tuple(alloc.tensor_shape)
            dtype = mybir.dt.np(alloc.dtype)
            out_avals.append(jax.core.ShapedArray(shape, dtype))
            zero_outs.append(_np.zeros(shape, dtype))
    n_params = len(in_names)
    n_outs = len(out_avals)
    all_in_names = list(in_names) + list(out_names)
    if partition_name is not None:
        all_in_names.append(partition_name)

    def _body(*args):
        operands = list(args)
        if partition_name is not None:
            operands.append(b2j.partition_id_tensor())
        outs = b2j._bass_exec_p.bind(
            *operands,
            out_avals=tuple(out_avals),
            in_names=tuple(all_in_names),
            out_names=tuple(out_names),
            lowering_input_output_aliases=(),
            sim_require_finite=True,
            sim_require_nnan=True,
            nc=nc_,
        )
        return tuple(outs)

    n_cores = len(in_maps)
    devices = jax.devices()[:n_cores]
    mesh = Mesh(_np.asarray(devices), ("core",))
    in_specs = (PartitionSpec("core"),) * (n_params + n_outs)
    out_specs = (PartitionSpec("core"),) * len(out_names)
    sharded = jax.jit(
        shard_map(_body, mesh=mesh, in_specs=in_specs, out_specs=out_specs,
                  check_rep=False),
        keep_unused=True,
    )
    per_core = [[_np.asarray(m[name]) for name in in_names] for m in in_maps]
    concat_in = [
        _np.concatenate([per_core[c][i] for c in range(n_cores)], axis=0)
        for i in range(n_params)
    ]
    concat_zeros = [
        _np.zeros((n_cores * z.shape[0], *z.shape[1:]), z.dtype) for z in zero_outs
    ]
    return sharded, concat_in, concat_zeros


def _build_sharded_chain(nc, in_maps, chain):
    """Like _build_sharded but executes the NEFF `chain` times per dispatch,
    serialised by threading the output buffers through as the donated
    zero-output operands."""
    import jax
    import numpy as _np
    from jax.sharding import Mesh, PartitionSpec
    from jax.experimental.shard_map import shard_map
    from concourse import bass2jax as b2j

    b2j.install_neuronx_cc_hook()
    nc_ = nc
    partition_name = nc_.partition_id_tensor.name if nc_.partition_id_tensor else None
    in_names, out_names, out_avals, zero_outs = [], [], [], []
    for alloc in nc_.m.functions[0].allocations:
        if not isinstance(alloc, mybir.MemoryLocationSet):
            continue
        name = alloc.memorylocations[0].name
        if alloc.kind == "ExternalInput":
            if name != partition_name:
                in_names.append(name)
        elif alloc.kind == "ExternalOutput":
            out_names.append(name)
            shape = tuple(alloc.tensor_shape)
            dtype = mybir.dt.np(alloc.dtype)
            out_avals.append(jax.core.ShapedArray(shape, dtype))
            zero_outs.append(_np.zeros(shape, dtype))
    n_params = len(in_names)
    all_in_names = list(in_names) + list(out_names)
    if partition_name is not None:
        all_in_names.append(partition_name)

    def _body(*args):
        ins = list(args[:n_params])
        outs = list(args[n_params:])
        for _ in range(chain):
            operands = ins + outs
            if partition_name is not None:
                operands.append(b2j.partition_id_tensor())
            outs = list(b2j._bass_exec_p.bind(
                *operands,
                out_avals=tuple(out_avals),
                in_names=tuple(all_in_names),
                out_names=tuple(out_names),
                lowering_input_output_aliases=(),
                sim_require_finite=True,
                sim_require_nnan=True,
                nc=nc_,
            ))
        return tuple(outs)

    n_cores = len(in_maps)
    devices = jax.devices()[:n_cores]
    mesh = Mesh(_np.asarray(devices), ("core",))
    in_specs = (PartitionSpec("core"),) * (n_params + len(out_names))
    out_specs = (PartitionSpec("core"),) * len(out_names)
    sharded = jax.jit(
        shard_map(_body, mesh=mesh, in_specs=in_specs, out_specs=out_specs,
                  check_rep=False),
        keep_unused=True,
    )
    per_core = [[_np.asarray(m[name]) for name in in_names] for m in in_maps]
    concat_in = [
        _np.concatenate([per_core[c][i] for c in range(n_cores)], axis=0)
        for i in range(n_params)
    ]
    concat_zeros = [
        _np.zeros((n_cores * z.shape[0], *z.shape[1:]), z.dtype) for z in zero_outs
    ]
    return sharded, concat_in, concat_zeros


def bench_chain(np_inputs, reps=6, chain_hi=5):
    """Marginal per-iteration device time via an in-NEFF repeat loop."""
    import jax, time
    from jax.sharding import Mesh, PartitionSpec, NamedSharding
    c_dt = _dt_matrix()
    in_maps = []
    for r in range(8):
        in_maps.append({
            "sig1": np.ascontiguousarray(np_inputs["reference_signal"][r].reshape(P, NCOL)),
            "sig2": np.ascontiguousarray(np_inputs["other_signal"][r].reshape(P, NCOL)),
            "c_dt": c_dt,
        })
    times = {}
    for chain in (1, chain_hi):
        nc = _get_nc(chain, **BEST_KW)
        fn, ci, cz = _build_sharded(nc, in_maps)
        mesh = Mesh(np.asarray(jax.devices()[:8]), ("core",))
        sh = NamedSharding(mesh, PartitionSpec("core"))
        dev_in = [jax.device_put(a, sh) for a in ci]
        dev_zero = [jax.device_put(a, sh) for a in cz]
        jax.block_until_ready(fn(*dev_in, *dev_zero))
        best = float("inf")
        for _ in range(reps):
            t0 = time.perf_counter()
            jax.block_until_ready(fn(*dev_in, *dev_zero))
            t1 = time.perf_counter()
            best = min(best, t1 - t0)
        times[chain] = best
    marg = (times[chain_hi] - times[1]) / (chain_hi - 1)
    return marg * 1e9, times


def bench(np_inputs, iters=30):
    import jax, time
    nc = _get_nc(1, **BEST_KW)
    c_dt = _dt_matrix()
    in_maps = []
    for r in range(8):
        in_maps.append({
            "sig1": np.ascontiguousarray(np_inputs["reference_signal"][r].reshape(P, NCOL)),
            "sig2": np.ascontiguousarray(np_inputs["other_signal"][r].reshape(P, NCOL)),
            "c_dt": c_dt,
        })
    fn, concat_in, concat_zeros = _build_sharded(nc, in_maps)
    from jax.sharding import Mesh, PartitionSpec, NamedSharding
    mesh = Mesh(np.asarray(jax.devices()[:8]), ("core",))
    sh = NamedSharding(mesh, PartitionSpec("core"))
    dev_in = [jax.device_put(a, sh) for a in concat_in]
    dev_zero = [jax.device_put(a, sh) for a in concat_zeros]
    jax.block_until_ready(fn(*dev_in, *dev_zero))  # warm/compile
    jax.block_until_ready(fn(*dev_in, *dev_zero))
    t0 = time.perf_counter()
    for _ in range(iters):
        out = fn(*dev_in, *dev_zero)
    jax.block_until_ready(out)
    t1 = time.perf_counter()
    return (t1 - t0) / iters * 1e9


if __name__ == "__main__":
    rng = np.random.default_rng(0)
    a = rng.random((8, 131072), np.float32)
    b = rng.random((8, 131072), np.float32)
    print(kernel(a, b))


def bench_marginal(np_inputs, ra=6, rb=16, rounds=8, iters=50):
    """Per-execution device time: slope of wall time vs in-NEFF repeat count,
    measured on a single core (identical per-core work), best-of interleaved
    rounds to cancel drift."""
    import jax, time
    from concourse import bass2jax as b2j
    c_dt = _dt_matrix()
    in_map = {"sig1": np.ascontiguousarray(np_inputs["reference_signal"][0].reshape(P, NCOL)),
              "sig2": np.ascontiguousarray(np_inputs["other_signal"][0].reshape(P, NCOL)),
              "c_dt": c_dt}

    def build_one(nc):
        b2j.install_neuronx_cc_hook()
        partition_name = nc.partition_id_tensor.name if nc.partition_id_tensor else None
        in_names, out_names, out_avals, zero_outs = [], [], [], []
        for alloc in nc.m.functions[0].allocations:
            if not isinstance(alloc, mybir.MemoryLocationSet):
                continue
            name = alloc.memorylocations[0].name
            if alloc.kind == "ExternalInput":
                if name != partition_name:
                    in_names.append(name)
            elif alloc.kind == "ExternalOutput":
                out_names.append(name)
                shape = tuple(alloc.tensor_shape)
                dtype = mybir.dt.np(alloc.dtype)
                out_avals.append(jax.core.ShapedArray(shape, dtype))
                zero_outs.append(np.zeros(shape, dtype))
        all_in = list(in_names) + list(out_names)
        if partition_name is not None:
            all_in.append(partition_name)

        def _body(*args):
            operands = list(args)
            if partition_name is not None:
                operands.append(b2j.partition_id_tensor())
            return tuple(b2j._bass_exec_p.bind(
                *operands, out_avals=tuple(out_avals), in_names=tuple(all_in),
                out_names=tuple(out_names), lowering_input_output_aliases=(),
                sim_require_finite=True, sim_require_nnan=True, nc=nc))

        fn = jax.jit(_body, keep_unused=True)
        args = [np.asarray(in_map[n]) for n in in_names] + zero_outs
        dargs = [jax.device_put(a, jax.devices()[0]) for a in args]
        return fn, dargs

    fns = {}
    for rep in (ra, rb):
        fn, dargs = build_one(build_nc(rep, **BEST_KW))
        jax.block_until_ready(fn(*dargs))
        fns[rep] = (fn, dargs)
    best = {rep: float("inf") for rep in fns}
    for _ in range(rounds):
        for rep, (fn, dargs) in fns.items():
            t0 = time.perf_counter()
            for _ in range(iters):
                out = fn(*dargs)
            jax.block_until_ready(out)
            t1 = time.perf_counter()
            best[rep] = min(best[rep], (t1 - t0) / iters)
    return (best[rb] - best[ra]) / (rb - ra) * 1e9

